# revision 19
# baseline (speedup 1.0000x reference)
"""EpiPINN loss kernel for 8 Trainium2 NeuronCores (Bass/Tile).

Computes: 6-layer tanh MLP (1->512x5->5) over 8192 collocation points,
softmax -> SEIRD components y, Caputo L1 fractional derivative (lower
triangular Toeplitz [8191x8191] @ dpsi), SEIRD residual, scalar MSE loss.

Distribution: data-parallel MLP over rows (1024/core); the Toeplitz matmul
computes per-core partial convolutions for all 64 output blocks from the
local dpsi (core-shifted band wmega); ReduceScatter sums them so core d
receives its own 8 blocks; per-core partial loss summed on host.

v4 schedule: ACT runs one table set (tanh+exp) for the whole kernel.
Layer 0 is an outer product, done as a partition-broadcast DMA of t plus
ACT `scale=`-fused multiplies (no PE, no f32r 2-pass). The Caputo band
(m^e) is evaluated on the DVE with frexp-bitcast + Horner polynomials;
the scalar params (softplus/lgamma/C) run the same polynomials on the
otherwise-idle GPSIMD. PE warm-up bursts use a memset tile (no DMA
dependency) so nothing early ever waits on a slow queue. Collective
payload is f16.
"""

import math

import numpy as np

H = 512
DEPTH = 6
N = 8192
DT = 0.1
MIN_ALPHA = 0.6
NCORES = 8
ROWS = N // NCORES          # 1024 rows per core
NB = N // 128               # 64 global 128-row blocks
NQ = NB // NCORES           # 8 out-blocks per core
WB = 8320                   # wbuf length = 128 * 65  (shifted-kernel values)
WBC = 65                    # wbuf free cols per partition
WMC = 128 * 64              # Wmega columns: diagonals m'' = 0..63
KT = H // 128               # 4 contraction tiles
RLOC = ROWS                 # 1024 rows per core
CH2 = ((0, 512), (512, 512))

COLLECTIVE = "A2A"          # "RS" or "A2A"
CC_DT = "f16"               # collective payload dtype

_CACHE = {}

_SQ2 = math.sqrt(2.0)
_LN2 = math.log(2.0)


def _lgamma_coeffs(deg=7):
    x = np.linspace(1.0, 1.4, 2001)
    y = np.array([math.lgamma(v) for v in x])
    return np.polyfit(x, y, deg)


def _ln_coeffs(deg=10):
    # ln(1+t) on [sqrt(2)/2-1, sqrt(2)-1]; max abs err ~2.4e-9
    t = np.linspace(_SQ2 / 2 - 1, _SQ2 - 1, 20001)
    return np.polyfit(t, np.log1p(t), deg)


def _exp_coeffs(deg=12):
    # e^x on [0, 3.65] centered at 1.825; max rel err ~2e-9
    x = np.linspace(0.0, 3.65, 20001)
    return np.polyfit(x - 1.825, np.exp(x), deg)


def _build():
    import concourse.bass as bass
    import concourse.tile as tile
    from concourse import bacc, mybir
    from ml_dtypes import bfloat16 as ml_bf16

    f32 = mybir.dt.float32
    bf16 = mybir.dt.bfloat16
    f16 = mybir.dt.float16
    i32 = mybir.dt.int32
    f32r = mybir.dt.float32r
    AF = mybir.ActivationFunctionType
    OP = mybir.AluOpType
    ccdt = f16 if CC_DT == "f16" else f32

    nc = bacc.Bacc("TRN2", target_bir_lowering=False, debug=False,
                   num_devices=NCORES)

    # ---- kernel I/O ----
    tsh = nc.dram_tensor("tsh", [128, RLOC], f32, kind="ExternalInput")
    # all small inputs packed into one tensor -> one DMA, one descgen
    smalls = nc.dram_tensor("smalls", [128, 44], f32, kind="ExternalInput")
    whp = nc.dram_tensor("whp", [128, (DEPTH - 1) * KT * H], f16,
                         kind="ExternalInput")
    woutp = nc.dram_tensor("woutp", [128, KT * 5], f16, kind="ExternalInput")
    out_d = nc.dram_tensor("out", [128, 1], f32, kind="ExternalOutput")

    j128_d = nc.inline_tensor(
        np.eye(128, dtype=np.float32)[::-1].copy().astype(ml_bf16),
        name="j128")

    lg = _lgamma_coeffs()
    lnc = _ln_coeffs()
    exc = _exp_coeffs()

    def ln_dve(eng, pool, out, x_ap, pcount, cols, tagp):
        """out = ln(x) for positive f32 x via frexp bitcast + Horner."""
        xi = x_ap.bitcast(i32)
        fi = pool.tile([pcount, cols], i32, tag=tagp + "fi")
        eng.tensor_scalar(fi[:], xi, 0x7FFFFF, 0x3F800000,
                          OP.bitwise_and, OP.bitwise_or)
        f = fi[:].bitcast(f32)
        ei = pool.tile([pcount, cols], i32, tag=tagp + "ei")
        eng.tensor_scalar(ei[:], xi, 0x7F800000, None, OP.bitwise_and)
        ef = pool.tile([pcount, cols], f32, tag=tagp + "ef")
        eng.tensor_copy(ef[:], ei[:])           # exact int -> f32
        eng.tensor_scalar(ef[:], ef[:], 2.0 ** -23, -127.0,
                          OP.mult, OP.add)
        msk = pool.tile([pcount, cols], f32, tag=tagp + "mk")
        eng.tensor_scalar(msk[:], f, -_SQ2, 1e30, OP.add, OP.mult)
        eng.tensor_scalar(msk[:], msk[:], 0.0, 1.0, OP.max, OP.min)
        # e2 = ef + msk ; f2 = f * (1 - 0.5*msk) ; t = f2 - 1
        eng.tensor_tensor(ef[:], ef[:], msk[:], OP.add)
        eng.tensor_scalar(msk[:], msk[:], -0.5, 1.0, OP.mult, OP.add)
        t = pool.tile([pcount, cols], f32, tag=tagp + "t")
        eng.tensor_tensor(t[:], f, msk[:], OP.mult)
        eng.tensor_scalar_add(t[:], t[:], -1.0)
        p = pool.tile([pcount, cols], f32, tag=tagp + "p")
        eng.memset(p[:], float(lnc[0]))
        for c in lnc[1:]:
            eng.tensor_tensor(p[:], p[:], t[:], OP.mult)
            eng.tensor_scalar_add(p[:], p[:], float(c))
        eng.scalar_tensor_tensor(out, ef[:], _LN2, p[:], OP.mult, OP.add)

    def exp_dve(eng, pool, out, x_ap, pcount, cols, tagp):
        """out = exp(x) for x in [0, 3.65] via centered Horner."""
        u = pool.tile([pcount, cols], f32, tag=tagp + "u")
        eng.tensor_scalar_add(u[:], x_ap, -1.825)
        p = pool.tile([pcount, cols], f32, tag=tagp + "q")
        eng.memset(p[:], float(exc[0]))
        for c in exc[1:-1]:
            eng.tensor_tensor(p[:], p[:], u[:], OP.mult)
            eng.tensor_scalar_add(p[:], p[:], float(c))
        eng.tensor_tensor(p[:], p[:], u[:], OP.mult)
        eng.tensor_scalar(out, p[:], float(exc[-1]), None, OP.add)

    with tile.TileContext(nc, num_cores=NCORES) as tc:
        with (
            tc.tile_pool(name="dram", bufs=1, space="DRAM") as dram,
            tc.tile_pool(name="const", bufs=1) as cpool,
            tc.tile_pool(name="acts", bufs=1) as apool,
            tc.tile_pool(name="small", bufs=1) as spool,
        ):
            # ------- DRAM scratch -------
            wbuf_dram = dram.tile([WB], bf16)
            cc2_in = dram.tile([128 * NCORES, 40], ccdt)
            rs_out = dram.tile([128, 40], ccdt)
            if COLLECTIVE == "A2A":
                a2a_out = dram.tile([128 * NCORES, 40], ccdt)

            # ------- DMAs: smalls packed on sync; weights on the scalar
            # queue; t-broadcast on gpsimd -------
            sm = cpool.tile([128, 44], f32)
            nc.sync.dma_start(sm[:], smalls.ap())
            winp_sb = sm[:, 0:4]
            binp_sb = sm[:, 4:8]
            bhp_sb = sm[:, 8:28]
            ident5_sb = sm[0:5, 28:33]
            bout5_sb = sm[0:5, 33:34]
            par_sb = sm[0:1, 34:42]
            coref_sb = sm[0:1, 40:44]
            j128_sb = cpool.tile([128, 128], bf16)
            nc.sync.dma_start(j128_sb[:], j128_d.ap())

            # t pre-broadcast host-side (layer-0 moving operand)
            tb = cpool.tile([128, RLOC], f32)
            nc.sync.dma_start(tb[:], tsh.ap())

            whp_sb = cpool.tile([128, (DEPTH - 1) * KT * H], f16)
            nc.scalar.dma_start(whp_sb[:], whp.ap())
            wh_sb = [whp_sb[:, l * KT * H:(l + 1) * KT * H]
                     for l in range(DEPTH - 1)]
            woutp_sb = cpool.tile([128, KT * 5], f16)
            nc.scalar.dma_start(woutp_sb[:], woutp.ap())

            # memset consts (DVE, ready instantly)
            wmm = cpool.tile([128, 64], bf16)
            nc.vector.memset(wmm[:], 0.25)
            ones5f = cpool.tile([5, 1], f32)
            nc.vector.memset(ones5f[:], 1.0)
            ones5 = cpool.tile([5, 1], f32r)
            nc.vector.tensor_copy(ones5[:], ones5f[:])
            ones1x5 = cpool.tile([1, 5], f32)
            nc.vector.memset(ones1x5[:], 1.0)
            ones128t = cpool.tile([1, 128], f32)
            nc.vector.memset(ones128t[:], 1.0)
            ones128 = cpool.tile([128, 1], f32)
            nc.vector.memset(ones128[:], 1.0)

            # ------- params: the only ACT use outside tanh/exp stream ----
            sp_e = spool.tile([1, 8], f32, tag="sp")
            nc.scalar.activation(sp_e[0:1, 0:4], par_sb[0:1, 0:4], AF.Exp)
            alp = spool.tile([1, 4], f32, tag="alp")
            nc.scalar.activation(alp[0:1, 0:1], par_sb[0:1, 4:5], AF.Exp,
                                 scale=-1.0)
            # alpha chain on DVE (fast, feeds ebp)
            nc.vector.tensor_scalar_add(alp[0:1, 0:1], alp[0:1, 0:1], 1.0)
            nc.vector.reciprocal(alp[0:1, 1:2], alp[0:1, 0:1])
            nc.vector.tensor_scalar(alp[0:1, 2:3], alp[0:1, 1:2],
                                    1.0 - MIN_ALPHA, MIN_ALPHA,
                                    OP.mult, OP.add)
            nc.vector.tensor_scalar(alp[0:1, 3:4], alp[0:1, 2:3],
                                    -1.0, 1.0, OP.mult, OP.add)  # e = 1-a

            e2 = spool.tile([1, 4], f32, tag="e2")
            nc.vector.tensor_copy(e2[0:1, 0:1], alp[0:1, 3:4])
            nc.vector.tensor_copy(e2[0:1, 1:3], coref_sb[0:1, 0:2])
            nc.vector.tensor_copy(e2[0:1, 3:4], coref_sb[0:1, 2:3])
            eb = cpool.tile([128, 4], f32)
            with tc.tile_pool(name="psum_pre", bufs=1, space="PSUM") as peb:
                ebp = peb.tile([128, 4], f32, tag="ebp")
                nc.tensor.matmul(ebp[:], ones128t[:], e2[0:1, :],
                                 start=True, stop=True)
                nc.vector.tensor_copy(eb[:], ebp[:])
            e128 = eb[:, 0:1]
            shiftm1 = eb[:, 3:4]
            lastc5 = eb[0:5, 2:3]

            # ------- Caputo band on DVE -------
            wtmp = tc.tile_pool(name="wtmp", bufs=1)
            with wtmp as wt:
                vi = wt.tile([128, 66], i32, tag="vi")
                nc.gpsimd.iota(vi[:], [[1, 66]], channel_multiplier=65)
                qraw = wt.tile([128, 66], f32, tag="qraw")
                nc.vector.tensor_copy(qraw[:], vi[:])
                nc.vector.tensor_scalar(qraw[:], qraw[:], shiftm1, None,
                                        OP.add)
                mk1 = wt.tile([128, WBC], f32, tag="mk1")
                nc.vector.tensor_scalar(mk1[:], qraw[:, 0:WBC], 1.0, None,
                                        OP.add)
                nc.vector.tensor_scalar(mk1[:], mk1[:], 0.0, 1.0, OP.max,
                                        OP.min)
                mk2 = wt.tile([128, WBC], f32, tag="mk2")
                nc.vector.tensor_scalar(mk2[:], qraw[:, 0:WBC], 0.0, 1.0,
                                        OP.max, OP.min)
                mk3 = wt.tile([128, WBC], f32, tag="mk3")
                nc.vector.tensor_scalar(mk3[:], qraw[:, 0:WBC], -1.0, 8191.0,
                                        OP.mult, OP.add)
                nc.vector.tensor_scalar(mk3[:], mk3[:], 0.0, 1.0, OP.max,
                                        OP.min)
                qc = wt.tile([128, 66], f32, tag="qc")
                nc.vector.tensor_scalar(qc[:], qraw[:], 1.0, None, OP.max)
                g = wt.tile([128, 66], f32, tag="g")
                ln_dve(nc.vector, wt, g[:], qc[:], 128, 66, "L")
                nc.vector.tensor_scalar(g[:], g[:], e128, None, OP.mult)
                ee = wt.tile([128, 66], f32, tag="ee")
                exp_dve(nc.vector, wt, ee[:], g[:], 128, 66, "X")
                w1 = wt.tile([128, WBC], f32, tag="w1")
                nc.vector.tensor_tensor(w1[:], ee[:, 1:66], mk1[:], OP.mult)
                nc.vector.tensor_tensor(mk2[:], ee[:, 0:WBC], mk2[:],
                                        OP.mult)
                nc.vector.tensor_tensor(w1[:], w1[:], mk2[:], OP.subtract)
                nc.vector.tensor_tensor(w1[:], w1[:], mk3[:], OP.mult)
                wbf = wt.tile([128, WBC], bf16, tag="wbf")
                nc.vector.tensor_copy(wbf[:], w1[:])
                nc.sync.dma_start(
                    wbuf_dram[:].rearrange("(p f) -> p f", p=128), wbf[:])

            wmega = cpool.tile([128, WMC], bf16)
            src = bass.AP(
                tensor=wbuf_dram[:].tensor, offset=1,
                ap=[[1, 128], [1, WMC]])
            nc.sync.dma_start(wmega[:], src)

            # ------- softplus / lgamma / C on DVE, after the band work ---
            ve = nc.vector
            ve.tensor_scalar_add(sp_e[0:1, 0:4], sp_e[0:1, 0:4], 1.0)
            sp = spool.tile([1, 8], f32, tag="sp2")
            ln_dve(ve, spool, sp[0:1, 0:4], sp_e[0:1, 0:4], 1, 4, "S")
            lgm = spool.tile([1, 2], f32, tag="lgm")
            ve.tensor_scalar_add(lgm[0:1, 1:2], alp[0:1, 3:4], 1.0)
            ve.memset(lgm[0:1, 0:1], float(lg[0]))
            for k in range(1, len(lg)):
                ve.tensor_tensor(lgm[0:1, 0:1], lgm[0:1, 0:1],
                                 lgm[0:1, 1:2], OP.mult)
                ve.tensor_scalar_add(lgm[0:1, 0:1], lgm[0:1, 0:1],
                                     float(lg[k]))
            cc_s = spool.tile([1, 2], f32, tag="ccs")
            ve.scalar_tensor_tensor(
                cc_s[0:1, 0:1], alp[0:1, 2:3], -math.log(DT), lgm[0:1, 0:1],
                OP.mult, OP.subtract)
            exp_dve(ve, spool, cc_s[0:1, 1:2], cc_s[0:1, 0:1], 1, 1, "C")

            sc16 = spool.tile([1, 16], f32, tag="sc16")
            ve.tensor_copy(sc16[0:1, 0:4], sp[0:1, 0:4])
            ve.tensor_tensor(sc16[0:1, 4:5], sp[0:1, 2:3],
                             sp[0:1, 3:4], OP.add)
            ve.tensor_scalar(sc16[0:1, 5:6], sp[0:1, 1:2], -1.0, None,
                             OP.mult)
            ve.tensor_scalar(sc16[0:1, 6:7], sc16[0:1, 4:5], -1.0, None,
                             OP.mult)
            ve.tensor_copy(sc16[0:1, 7:8], cc_s[0:1, 1:2])
            scb = cpool.tile([128, 8], f32)
            nc.gpsimd.partition_broadcast(scb[:], sc16[0:1, 0:8])
            beta128 = scb[:, 0:1]
            sig128 = scb[:, 1:2]
            gam128 = scb[:, 2:3]
            mu128 = scb[:, 3:4]
            nsig128 = scb[:, 5:6]
            ngpm128 = scb[:, 6:7]
            c128 = scb[:, 7:8]

            # ------- MLP: L0 fused into ACT via scale=; L1+ on PE -------
            hT = [apool.tile([128, KT * RLOC], f16, tag="hA", name="hA"),
                  apool.tile([128, KT * RLOC], f16, tag="hB", name="hB")]
            for mt in range(KT):
                for c0, cw in CH2:
                    nc.scalar.activation(
                        hT[0][:, mt * RLOC + c0:mt * RLOC + c0 + cw],
                        tb[:, c0:c0 + cw], AF.Tanh,
                        scale=winp_sb[:, mt:mt + 1],
                        bias=binp_sb[:, mt:mt + 1])
            # PE warm-up right before the hidden-layer stream so the HAM
            # clock gate is released when L1 starts (memset tile: no DMA dep)
            with tc.tile_pool(name="psum_warm", bufs=1, space="PSUM") as pw:
                warm = pw.tile([64, 64], f32, tag="warm")
                for wi in range(45):
                    nc.tensor.matmul(
                        warm[:], wmm[:, 0:64], wmm[:, 0:64],
                        start=(wi == 0), stop=(wi == 44))
            with tc.tile_pool(name="psum_mlp", bufs=1, space="PSUM") as pmm:
                for l in range(DEPTH - 1):
                    src_t, dst_t = hT[l % 2], hT[(l + 1) % 2]
                    for c0, cw in CH2:
                        for mt in range(KT):
                            ps = pmm.tile([128, 512], f32, tag="mlp",
                                          name="ps", bufs=4)
                            for kt in range(KT):
                                nc.tensor.matmul(
                                    ps[:, 0:cw],
                                    wh_sb[l][:, kt * H + mt * 128:
                                             kt * H + mt * 128 + 128],
                                    src_t[:, kt * RLOC + c0:
                                          kt * RLOC + c0 + cw],
                                    start=(kt == 0), stop=(kt == KT - 1))
                            nc.scalar.activation(
                                dst_t[:, mt * RLOC + c0:mt * RLOC + c0 + cw],
                                ps[:, 0:cw], AF.Tanh,
                                bias=bhp_sb[:, l * KT + mt:l * KT + mt + 1])

                # output layer + per-chunk softmax pipeline
                hlast = hT[(DEPTH - 1) % 2]
                ezT = apool.tile([5, RLOC], f32r, tag="ezT")
                rinv = apool.tile([1, RLOC], f32, tag="rinv")
                rscr = apool.tile([1, RLOC], f32, tag="rscr")
                yT = apool.tile([5, RLOC], f32, tag="yT")
                dpsiT = apool.tile([5, ROWS], f32, tag="dpsiT")
                for ci, (c0, cw) in enumerate(CH2):
                    ps = pmm.tile([5, 512], f32, tag="zed", name="ps", bufs=1)
                    for kt in range(KT):
                        nc.tensor.matmul(
                            ps[:, 0:cw],
                            woutp_sb[:, kt * 5:(kt + 1) * 5],
                            hlast[:, kt * RLOC + c0:kt * RLOC + c0 + cw],
                            start=(kt == 0), stop=(kt == KT - 1))
                    nc.scalar.activation(
                        ezT[:, c0:c0 + cw], ps[:, 0:cw], AF.Exp,
                        bias=bout5_sb[:, 0:1])
                    pss = pmm.tile([1, 512], f32, tag="ssum", name="ps",
                                   bufs=2)
                    nc.tensor.matmul(
                        pss[:, 0:cw], ones5[:], ezT[:, c0:c0 + cw],
                        start=True, stop=True)
                    nc.vector.reciprocal_approx_accurate(
                        rinv[0:1, c0:c0 + cw], pss[0:1, 0:cw],
                        rscr[0:1, c0:c0 + cw])
                    # keep PE HAM-warm while DVE runs the reciprocal
                    wz = pmm.tile([128, 512], f32, tag="mlp", name="ps",
                                  bufs=4)
                    for wi in range(25):
                        nc.tensor.matmul(
                            wz[0:64, 0:64], wmm[:, 0:64], wmm[:, 0:64],
                            start=(wi == 0), stop=(wi == 24))
                    psr = pmm.tile([5, 512], f32, tag="rrep", name="ps",
                                   bufs=1)
                    nc.tensor.matmul(
                        psr[:, 0:cw], ones1x5[:], rinv[0:1, c0:c0 + cw],
                        start=True, stop=True)
                    nc.vector.tensor_tensor(
                        yT[:, c0:c0 + cw],
                        ezT[:, c0:c0 + cw], psr[:, 0:cw], OP.mult)
                    lo = c0 - 1 if ci else 0
                    hi = c0 + cw - 1
                    nc.vector.tensor_tensor(
                        dpsiT[:, lo:hi], yT[:, lo + 1:hi + 1],
                        yT[:, lo:hi], OP.subtract)

            nc.vector.tensor_scalar(dpsiT[:, ROWS - 1:ROWS],
                                    dpsiT[:, ROWS - 2:ROWS - 1],
                                    lastc5, None, OP.mult)

            # keep PE busy through the DVE softmax tail so the HAM gate
            # stays open for the fold/rev/conv burst
            with tc.tile_pool(name="psum_w3", bufs=1, space="PSUM") as pw3:
                wz3 = pw3.tile([64, 64], f32, tag="warm3")
                for wi in range(50):
                    nc.tensor.matmul(
                        wz3[:], wmm[:, 0:64], wmm[:, 0:64],
                        start=(wi == 0), stop=(wi == 49))

            # ------- fold dpsi only (critical path to the collective) ----
            dloc = spool.tile([128, 40], bf16, tag="dloc")
            yloc = spool.tile([128, 40], f32, tag="yloc")
            with tc.tile_pool(name="psum_fold", bufs=1,
                              space="PSUM") as pf:
                ptd = pf.tile([128, 40], f32, tag="fold")
                for j in range(NQ):
                    nc.tensor.transpose(
                        ptd[:, j * 5:(j + 1) * 5],
                        dpsiT[:, j * 128:(j + 1) * 128],
                        ident5_sb[:],
                    )
                nc.vector.tensor_copy(dloc[:], ptd[:])

                dgr = spool.tile([128, 40], bf16, tag="dgr")
                pr = pf.tile([128, 40], f32, tag="rev")
                nc.tensor.matmul(pr[:], j128_sb[:], dloc[:],
                                 start=True, stop=True)
                nc.vector.tensor_copy(dgr[:], pr[:])
            # ------- local partial Toeplitz conv over all 64 blocks ------
            with tc.tile_pool(name="psum_out", bufs=2, space="PSUM") as po:
                conv = po.tile([128, NB * 5], f32, tag="conv")
                ms = list(range(0, NB, NQ)) + [m for m in range(NB)
                                               if m % NQ != 0]
                for i, m in enumerate(ms):
                    nj = min(NQ, NB - m)
                    nc.tensor.matmul(
                        conv[:, 5 * m:5 * (m + nj)],
                        wmega[:, 128 * m:128 * (m + 1)],
                        dgr[:, 0:5 * nj],
                        start=(i == 0), stop=(i == len(ms) - 1))
                conv_sb = spool.tile([128, NB * 5], ccdt, tag="convsb")
                nc.scalar.copy(conv_sb[:], conv[:])   # idle ACT engine
                nc.gpsimd.dma_start(
                    cc2_in[:].rearrange("(g p) f -> p g f", p=128),
                    conv_sb[:].rearrange("p (g f) -> p g f", g=NCORES))

            # ------- y fold + f (overlap the collective wait) -------
            with tc.tile_pool(name="psum_yfold", bufs=1,
                              space="PSUM") as pfy:
                pty = pfy.tile([128, 40], f32, tag="yfold")
                for j in range(NQ):
                    nc.tensor.transpose(
                        pty[:, j * 5:(j + 1) * 5],
                        yT[:, j * 128:(j + 1) * 128],
                        ident5_sb[:],
                    )
                nc.vector.tensor_copy(yloc[:], pty[:])
            yb4 = yloc[:].rearrange("p (q c) -> p q c", q=NQ)
            fb = spool.tile([128, 40], f32, tag="fb")
            fb4 = fb[:].rearrange("p (q c) -> p q c", q=NQ)
            t1 = spool.tile([128, NQ], f32, tag="t1")
            liv = spool.tile([128, NQ], f32, tag="liv")
            nc.vector.tensor_scalar(liv[:], yb4[:, :, 4], -1.0, 1.0,
                                    OP.mult, OP.add)
            nc.vector.reciprocal(liv[:], liv[:])
            nc.vector.tensor_tensor(t1[:], yb4[:, :, 0], yb4[:, :, 2],
                                    OP.mult)
            nc.vector.tensor_tensor(t1[:], t1[:], liv[:], OP.mult)
            nc.vector.tensor_scalar(t1[:], t1[:], beta128, None, OP.mult)
            nc.vector.tensor_scalar(fb4[:, :, 0], t1[:], -1.0, None,
                                    OP.mult)
            nc.vector.scalar_tensor_tensor(
                fb4[:, :, 1], yb4[:, :, 1], nsig128, t1[:],
                OP.mult, OP.add)
            nc.vector.tensor_scalar(t1[:], yb4[:, :, 1], sig128, None,
                                    OP.mult)
            nc.vector.scalar_tensor_tensor(
                fb4[:, :, 2], yb4[:, :, 2], ngpm128, t1[:],
                OP.mult, OP.add)
            nc.vector.tensor_scalar(fb4[:, :, 3], yb4[:, :, 2], gam128,
                                    None, OP.mult)
            nc.vector.tensor_scalar(fb4[:, :, 4], yb4[:, :, 2], mu128,
                                    None, OP.mult)

            # ------- collective: sum partial convs across cores ----------
            rsb = spool.tile([128, 40], f32, tag="rsb")
            if COLLECTIVE == "RS":
                nc.gpsimd.collective_compute(
                    "ReduceScatter", OP.add,
                    replica_groups=[list(range(NCORES))],
                    ins=[cc2_in[:].opt()], outs=[rs_out[:].opt()])
                nc.gpsimd.dma_start(rsb[:], rs_out[:])
            else:
                nc.gpsimd.collective_compute(
                    "AllToAll", OP.bypass,
                    replica_groups=[list(range(NCORES))],
                    ins=[cc2_in[:].opt()], outs=[a2a_out[:].opt()])
                rsb8 = spool.tile([128, NCORES * 40], ccdt, tag="rsb8")
                nc.gpsimd.dma_start(
                    rsb8[:].rearrange("p (s f) -> p s f", s=NCORES),
                    a2a_out[:].rearrange("(s p) f -> p s f", p=128))
                a1 = spool.tile([128, 160], f32, tag="a1")
                nc.vector.tensor_tensor(a1[:], rsb8[:, 0:160],
                                        rsb8[:, 160:320], OP.add)
                nc.vector.tensor_tensor(a1[:, 0:80], a1[:, 0:80],
                                        a1[:, 80:160], OP.add)
                nc.vector.tensor_tensor(rsb[:], a1[:, 0:40],
                                        a1[:, 40:80], OP.add)

            # ------- residual + per-row partial loss (host reduces) ------
            res = spool.tile([128, 40], f32, tag="res")
            nc.vector.scalar_tensor_tensor(res[:], rsb[:], c128, fb[:],
                                           OP.mult, OP.subtract)
            sq = spool.tile([128, 40], f32, tag="sq")
            rowsum = spool.tile([128, 1], f32, tag="rowsum")
            nc.vector.scalar_tensor_tensor(
                sq[:], res[:], 0.0, res[:], OP.add, OP.mult,
                accum_out=rowsum[:])
            nc.sync.dma_start(out_d.ap(), rowsum[:])

    nc.compile()
    return nc


def _in_maps(inputs):
    t = np.asarray(inputs["t"], np.float32)
    W_in = np.asarray(inputs["W_in"], np.float32)
    b_in = np.asarray(inputs["b_in"], np.float32)
    Wh = np.asarray(inputs["Wh"], np.float32)
    bh = np.asarray(inputs["bh"], np.float32)
    W_out = np.asarray(inputs["W_out"], np.float32)
    b_out = np.asarray(inputs["b_out"], np.float32)

    whp = np.ascontiguousarray(
        Wh.reshape(DEPTH - 1, KT, 128, H).transpose(2, 0, 1, 3)
        .reshape(128, (DEPTH - 1) * KT * H)).astype(np.float16)
    woutp = np.ascontiguousarray(
        W_out.reshape(KT, 128, 5).transpose(1, 0, 2)
        .reshape(128, KT * 5)).astype(np.float16)

    base = np.zeros((128, 44), np.float32)
    base[:, 0:4] = W_in.reshape(KT, 128).T
    base[:, 4:8] = b_in.reshape(KT, 128).T
    base[:, 8:28] = (bh.reshape(DEPTH - 1, KT, 128).transpose(2, 0, 1)
                     .reshape(128, (DEPTH - 1) * KT))
    base[0:5, 28:33] = np.eye(5, dtype=np.float32)
    base[0:5, 33] = b_out
    base[0, 34] = inputs["raw_beta"][0]
    base[0, 35] = inputs["raw_sigma"][0]
    base[0, 36] = inputs["raw_gamma"][0]
    base[0, 37] = inputs["raw_mu"][0]
    base[0, 38] = inputs["z_alpha"][0]

    maps = []
    for d in range(NCORES):
        shift = -128.0 - 1024.0 * d
        smalls = base.copy()
        smalls[0, 40] = shift
        smalls[0, 41] = 1.0 if d < NCORES - 1 else 0.0
        smalls[0, 42] = shift - 1.0
        maps.append({
            "tsh": np.ascontiguousarray(
                np.broadcast_to(t[d * ROWS:(d + 1) * ROWS, 0].reshape(
                    1, RLOC), (128, RLOC))),
            "smalls": smalls,
            "whp": whp,
            "woutp": woutp,
        })
    return maps


def kernel(**inputs) -> np.ndarray:
    from concourse.bass_utils import run_bass_kernel_spmd

    if "nc" not in _CACHE:
        _CACHE["nc"] = _build()
    nc = _CACHE["nc"]
    res = run_bass_kernel_spmd(nc, _in_maps(inputs), list(range(NCORES)))
    total = np.float32(0.0)
    for r in res.results:
        part = np.float32(np.asarray(r["out"], np.float32).sum())
        total = np.float32(total + np.float32(part / np.float32(N * 5)))
    return np.asarray(total, np.float32).reshape(())


# revision 21
# speedup vs baseline: 1.0429x; 1.0429x over previous
"""EpiPINN loss kernel for 8 Trainium2 NeuronCores (Bass/Tile).

Computes: 6-layer tanh MLP (1->512x5->5) over 8192 collocation points,
softmax -> SEIRD components y, Caputo L1 fractional derivative (lower
triangular Toeplitz [8191x8191] @ dpsi), SEIRD residual, scalar MSE loss.

Distribution: data-parallel MLP over rows (1024/core); the Toeplitz matmul
computes per-core partial convolutions for all 64 output blocks from the
local dpsi (core-shifted band wmega); ReduceScatter sums them so core d
receives its own 8 blocks; per-core partial loss summed on host.

v4 schedule: ACT runs one table set (tanh+exp) for the whole kernel.
Layer 0 is an outer product, done as a partition-broadcast DMA of t plus
ACT `scale=`-fused multiplies (no PE, no f32r 2-pass). The Caputo band
(m^e) is evaluated on the DVE with frexp-bitcast + Horner polynomials;
the scalar params (softplus/lgamma/C) run the same polynomials on the
otherwise-idle GPSIMD. PE warm-up bursts use a memset tile (no DMA
dependency) so nothing early ever waits on a slow queue. Collective
payload is f16.
"""

import math

import numpy as np

H = 512
DEPTH = 6
N = 8192
DT = 0.1
MIN_ALPHA = 0.6
NCORES = 8
ROWS = N // NCORES          # 1024 rows per core
NB = N // 128               # 64 global 128-row blocks
NQ = NB // NCORES           # 8 out-blocks per core
WB = 8320                   # wbuf length = 128 * 65  (shifted-kernel values)
WBC = 65                    # wbuf free cols per partition
WMC = 128 * 64              # Wmega columns: diagonals m'' = 0..63
KT = H // 128               # 4 contraction tiles
RLOC = ROWS                 # 1024 rows per core
CH2 = ((0, 512), (512, 512))

COLLECTIVE = "A2A"          # "RS" or "A2A"
CC_DT = "f16"               # collective payload dtype

_CACHE = {}

_SQ2 = math.sqrt(2.0)
_LN2 = math.log(2.0)


def _lgamma_coeffs(deg=7):
    x = np.linspace(1.0, 1.4, 2001)
    y = np.array([math.lgamma(v) for v in x])
    return np.polyfit(x, y, deg)


def _ln_coeffs(deg=10):
    # ln(1+t) on [sqrt(2)/2-1, sqrt(2)-1]; max abs err ~2.4e-9
    t = np.linspace(_SQ2 / 2 - 1, _SQ2 - 1, 20001)
    return np.polyfit(t, np.log1p(t), deg)


def _exp_coeffs(deg=12):
    # e^x on [0, 3.65] centered at 1.825; max rel err ~2e-9
    x = np.linspace(0.0, 3.65, 20001)
    return np.polyfit(x - 1.825, np.exp(x), deg)


def _build():
    import concourse.bass as bass
    import concourse.tile as tile
    from concourse import bacc, mybir
    from ml_dtypes import bfloat16 as ml_bf16

    f32 = mybir.dt.float32
    bf16 = mybir.dt.bfloat16
    f16 = mybir.dt.float16
    i32 = mybir.dt.int32
    f32r = mybir.dt.float32r
    AF = mybir.ActivationFunctionType
    OP = mybir.AluOpType
    ccdt = f16 if CC_DT == "f16" else f32

    nc = bacc.Bacc("TRN2", target_bir_lowering=False, debug=False,
                   num_devices=NCORES)

    # ---- kernel I/O ----
    tsh = nc.dram_tensor("tsh", [128, RLOC], f32, kind="ExternalInput")
    # all small inputs packed into one tensor -> one DMA, one descgen
    smalls = nc.dram_tensor("smalls", [128, 44], f32, kind="ExternalInput")
    whp = nc.dram_tensor("whp", [128, (DEPTH - 1) * KT * H], f16,
                         kind="ExternalInput")
    woutp = nc.dram_tensor("woutp", [128, KT * 5], f16, kind="ExternalInput")
    out_d = nc.dram_tensor("out", [128, 1], f32, kind="ExternalOutput")

    j128_d = nc.inline_tensor(
        np.eye(128, dtype=np.float32)[::-1].copy().astype(ml_bf16),
        name="j128")

    lg = _lgamma_coeffs()
    lnc = _ln_coeffs()
    exc = _exp_coeffs()

    def ln_dve(eng, pool, out, x_ap, pcount, cols, tagp):
        """out = ln(x) for positive f32 x via frexp bitcast + Horner."""
        xi = x_ap.bitcast(i32)
        fi = pool.tile([pcount, cols], i32, tag=tagp + "fi")
        eng.tensor_scalar(fi[:], xi, 0x7FFFFF, 0x3F800000,
                          OP.bitwise_and, OP.bitwise_or)
        f = fi[:].bitcast(f32)
        ei = pool.tile([pcount, cols], i32, tag=tagp + "ei")
        eng.tensor_scalar(ei[:], xi, 0x7F800000, None, OP.bitwise_and)
        ef = pool.tile([pcount, cols], f32, tag=tagp + "ef")
        eng.tensor_copy(ef[:], ei[:])           # exact int -> f32
        eng.tensor_scalar(ef[:], ef[:], 2.0 ** -23, -127.0,
                          OP.mult, OP.add)
        msk = pool.tile([pcount, cols], f32, tag=tagp + "mk")
        eng.tensor_scalar(msk[:], f, -_SQ2, 1e30, OP.add, OP.mult)
        eng.tensor_scalar(msk[:], msk[:], 0.0, 1.0, OP.max, OP.min)
        # e2 = ef + msk ; f2 = f * (1 - 0.5*msk) ; t = f2 - 1
        eng.tensor_tensor(ef[:], ef[:], msk[:], OP.add)
        eng.tensor_scalar(msk[:], msk[:], -0.5, 1.0, OP.mult, OP.add)
        t = pool.tile([pcount, cols], f32, tag=tagp + "t")
        eng.tensor_tensor(t[:], f, msk[:], OP.mult)
        eng.tensor_scalar_add(t[:], t[:], -1.0)
        p = pool.tile([pcount, cols], f32, tag=tagp + "p")
        eng.memset(p[:], float(lnc[0]))
        for c in lnc[1:]:
            eng.tensor_tensor(p[:], p[:], t[:], OP.mult)
            eng.tensor_scalar_add(p[:], p[:], float(c))
        eng.scalar_tensor_tensor(out, ef[:], _LN2, p[:], OP.mult, OP.add)

    def exp_dve(eng, pool, out, x_ap, pcount, cols, tagp):
        """out = exp(x) for x in [0, 3.65] via centered Horner."""
        u = pool.tile([pcount, cols], f32, tag=tagp + "u")
        eng.tensor_scalar_add(u[:], x_ap, -1.825)
        p = pool.tile([pcount, cols], f32, tag=tagp + "q")
        eng.memset(p[:], float(exc[0]))
        for c in exc[1:-1]:
            eng.tensor_tensor(p[:], p[:], u[:], OP.mult)
            eng.tensor_scalar_add(p[:], p[:], float(c))
        eng.tensor_tensor(p[:], p[:], u[:], OP.mult)
        eng.tensor_scalar(out, p[:], float(exc[-1]), None, OP.add)

    with tile.TileContext(nc, num_cores=NCORES) as tc:
        with (
            tc.tile_pool(name="dram", bufs=1, space="DRAM") as dram,
            tc.tile_pool(name="const", bufs=1) as cpool,
            tc.tile_pool(name="acts", bufs=1) as apool,
            tc.tile_pool(name="small", bufs=1) as spool,
        ):
            # ------- DRAM scratch -------
            wbuf_dram = dram.tile([WB], bf16)
            cc2_in = dram.tile([128 * NCORES, 40], ccdt)
            rs_out = dram.tile([128, 40], ccdt)
            if COLLECTIVE == "A2A":
                a2a_out = dram.tile([128 * NCORES, 40], ccdt)

            # ------- DMAs: smalls packed on sync; weights on the scalar
            # queue; t-broadcast on gpsimd -------
            sm = cpool.tile([128, 44], f32)
            nc.sync.dma_start(sm[:], smalls.ap())
            winp_sb = sm[:, 0:4]
            binp_sb = sm[:, 4:8]
            bhp_sb = sm[:, 8:28]
            ident5_sb = sm[0:5, 28:33]
            bout5_sb = sm[0:5, 33:34]
            par_sb = sm[0:1, 34:42]
            coref_sb = sm[0:1, 40:44]
            j128_sb = cpool.tile([128, 128], bf16)
            nc.sync.dma_start(j128_sb[:], j128_d.ap())

            # t pre-broadcast host-side (layer-0 moving operand)
            tb = cpool.tile([128, RLOC], f32)
            nc.sync.dma_start(tb[:], tsh.ap())

            whp_sb = cpool.tile([128, (DEPTH - 1) * KT * H], f16)
            nc.scalar.dma_start(whp_sb[:], whp.ap())
            wh_sb = [whp_sb[:, l * KT * H:(l + 1) * KT * H]
                     for l in range(DEPTH - 1)]
            woutp_sb = cpool.tile([128, KT * 5], f16)
            nc.scalar.dma_start(woutp_sb[:], woutp.ap())

            # memset consts (DVE, ready instantly)
            wmm = cpool.tile([128, 64], bf16)
            nc.vector.memset(wmm[:], 0.25)
            ones5f = cpool.tile([5, 1], f32)
            nc.vector.memset(ones5f[:], 1.0)
            ones5 = cpool.tile([5, 1], f32r)
            nc.vector.tensor_copy(ones5[:], ones5f[:])
            ones1x5 = cpool.tile([1, 5], f32)
            nc.vector.memset(ones1x5[:], 1.0)
            ones128t = cpool.tile([1, 128], f32)
            nc.vector.memset(ones128t[:], 1.0)
            ones128 = cpool.tile([128, 1], f32)
            nc.vector.memset(ones128[:], 1.0)

            # ------- params: the only ACT use outside tanh/exp stream ----
            sp_e = spool.tile([1, 8], f32, tag="sp")
            nc.scalar.activation(sp_e[0:1, 0:4], par_sb[0:1, 0:4], AF.Exp)
            alp = spool.tile([1, 4], f32, tag="alp")
            nc.scalar.activation(alp[0:1, 0:1], par_sb[0:1, 4:5], AF.Exp,
                                 scale=-1.0)
            # alpha chain on DVE (fast, feeds ebp)
            nc.vector.tensor_scalar_add(alp[0:1, 0:1], alp[0:1, 0:1], 1.0)
            nc.vector.reciprocal(alp[0:1, 1:2], alp[0:1, 0:1])
            nc.vector.tensor_scalar(alp[0:1, 2:3], alp[0:1, 1:2],
                                    1.0 - MIN_ALPHA, MIN_ALPHA,
                                    OP.mult, OP.add)
            nc.vector.tensor_scalar(alp[0:1, 3:4], alp[0:1, 2:3],
                                    -1.0, 1.0, OP.mult, OP.add)  # e = 1-a

            e2 = spool.tile([1, 4], f32, tag="e2")
            nc.vector.tensor_copy(e2[0:1, 0:1], alp[0:1, 3:4])
            nc.vector.tensor_copy(e2[0:1, 1:3], coref_sb[0:1, 0:2])
            nc.vector.tensor_copy(e2[0:1, 3:4], coref_sb[0:1, 2:3])
            eb = cpool.tile([128, 4], f32)
            with tc.tile_pool(name="psum_pre", bufs=1, space="PSUM") as peb:
                ebp = peb.tile([128, 4], f32, tag="ebp")
                nc.tensor.matmul(ebp[:], ones128t[:], e2[0:1, :],
                                 start=True, stop=True)
                nc.vector.tensor_copy(eb[:], ebp[:])
            e128 = eb[:, 0:1]
            shiftm1 = eb[:, 3:4]
            lastc5 = eb[0:5, 2:3]

            # ------- Caputo band on DVE -------
            wtmp = tc.tile_pool(name="wtmp", bufs=1)
            with wtmp as wt:
                vi = wt.tile([128, 66], i32, tag="vi")
                nc.gpsimd.iota(vi[:], [[1, 66]], channel_multiplier=65)
                qraw = wt.tile([128, 66], f32, tag="qraw")
                nc.vector.tensor_copy(qraw[:], vi[:])
                nc.vector.tensor_scalar(qraw[:], qraw[:], shiftm1, None,
                                        OP.add)
                mk1 = wt.tile([128, WBC], f32, tag="mk1")
                nc.vector.tensor_scalar(mk1[:], qraw[:, 0:WBC], 1.0, None,
                                        OP.add)
                nc.vector.tensor_scalar(mk1[:], mk1[:], 0.0, 1.0, OP.max,
                                        OP.min)
                mk2 = wt.tile([128, WBC], f32, tag="mk2")
                nc.vector.tensor_scalar(mk2[:], qraw[:, 0:WBC], 0.0, 1.0,
                                        OP.max, OP.min)
                mk3 = wt.tile([128, WBC], f32, tag="mk3")
                nc.vector.tensor_scalar(mk3[:], qraw[:, 0:WBC], -1.0, 8191.0,
                                        OP.mult, OP.add)
                nc.vector.tensor_scalar(mk3[:], mk3[:], 0.0, 1.0, OP.max,
                                        OP.min)
                qc = wt.tile([128, 66], f32, tag="qc")
                nc.vector.tensor_scalar(qc[:], qraw[:], 1.0, None, OP.max)
                g = wt.tile([128, 66], f32, tag="g")
                ln_dve(nc.vector, wt, g[:], qc[:], 128, 66, "L")
                nc.vector.tensor_scalar(g[:], g[:], e128, None, OP.mult)
                ee = wt.tile([128, 66], f32, tag="ee")
                exp_dve(nc.vector, wt, ee[:], g[:], 128, 66, "X")
                w1 = wt.tile([128, WBC], f32, tag="w1")
                nc.vector.tensor_tensor(w1[:], ee[:, 1:66], mk1[:], OP.mult)
                nc.vector.tensor_tensor(mk2[:], ee[:, 0:WBC], mk2[:],
                                        OP.mult)
                nc.vector.tensor_tensor(w1[:], w1[:], mk2[:], OP.subtract)
                nc.vector.tensor_tensor(w1[:], w1[:], mk3[:], OP.mult)
                wbf = wt.tile([128, WBC], bf16, tag="wbf")
                nc.vector.tensor_copy(wbf[:], w1[:])
                nc.sync.dma_start(
                    wbuf_dram[:].rearrange("(p f) -> p f", p=128), wbf[:])

            wmega = cpool.tile([128, WMC], bf16)
            src = bass.AP(
                tensor=wbuf_dram[:].tensor, offset=1,
                ap=[[1, 128], [1, WMC]])
            nc.sync.dma_start(wmega[:], src)

            # ------- softplus / lgamma / C on DVE, after the band work ---
            ve = nc.vector
            ve.tensor_scalar_add(sp_e[0:1, 0:4], sp_e[0:1, 0:4], 1.0)
            sp = spool.tile([1, 8], f32, tag="sp2")
            ln_dve(ve, spool, sp[0:1, 0:4], sp_e[0:1, 0:4], 1, 4, "S")
            lgm = spool.tile([1, 2], f32, tag="lgm")
            ve.tensor_scalar_add(lgm[0:1, 1:2], alp[0:1, 3:4], 1.0)
            ve.memset(lgm[0:1, 0:1], float(lg[0]))
            for k in range(1, len(lg)):
                ve.tensor_tensor(lgm[0:1, 0:1], lgm[0:1, 0:1],
                                 lgm[0:1, 1:2], OP.mult)
                ve.tensor_scalar_add(lgm[0:1, 0:1], lgm[0:1, 0:1],
                                     float(lg[k]))
            cc_s = spool.tile([1, 2], f32, tag="ccs")
            ve.scalar_tensor_tensor(
                cc_s[0:1, 0:1], alp[0:1, 2:3], -math.log(DT), lgm[0:1, 0:1],
                OP.mult, OP.subtract)
            exp_dve(ve, spool, cc_s[0:1, 1:2], cc_s[0:1, 0:1], 1, 1, "C")

            sc16 = spool.tile([1, 16], f32, tag="sc16")
            ve.tensor_copy(sc16[0:1, 0:4], sp[0:1, 0:4])
            ve.tensor_tensor(sc16[0:1, 4:5], sp[0:1, 2:3],
                             sp[0:1, 3:4], OP.add)
            ve.tensor_scalar(sc16[0:1, 5:6], sp[0:1, 1:2], -1.0, None,
                             OP.mult)
            ve.tensor_scalar(sc16[0:1, 6:7], sc16[0:1, 4:5], -1.0, None,
                             OP.mult)
            ve.tensor_copy(sc16[0:1, 7:8], cc_s[0:1, 1:2])
            scb = cpool.tile([128, 8], f32)
            nc.gpsimd.partition_broadcast(scb[:], sc16[0:1, 0:8])
            beta128 = scb[:, 0:1]
            sig128 = scb[:, 1:2]
            gam128 = scb[:, 2:3]
            mu128 = scb[:, 3:4]
            nsig128 = scb[:, 5:6]
            ngpm128 = scb[:, 6:7]
            c128 = scb[:, 7:8]

            # ------- MLP: L0 fused into ACT via scale=; L1+ on PE -------
            hT = [apool.tile([128, KT * RLOC], f16, tag="hA", name="hA"),
                  apool.tile([128, KT * RLOC], f16, tag="hB", name="hB")]
            for mt in range(KT):
                for c0, cw in CH2:
                    nc.scalar.activation(
                        hT[0][:, mt * RLOC + c0:mt * RLOC + c0 + cw],
                        tb[:, c0:c0 + cw], AF.Tanh,
                        scale=winp_sb[:, mt:mt + 1],
                        bias=binp_sb[:, mt:mt + 1])
            # PE warm-up right before the hidden-layer stream so the HAM
            # clock gate is released when L1 starts (memset tile: no DMA dep)
            with tc.tile_pool(name="psum_warm", bufs=1, space="PSUM") as pw:
                warm = pw.tile([64, 64], f32, tag="warm")
                for wi in range(45):
                    nc.tensor.matmul(
                        warm[:], wmm[:, 0:64], wmm[:, 0:64],
                        start=(wi == 0), stop=(wi == 44))
            hlast = hT[(DEPTH - 1) % 2]
            ezT = apool.tile([5, RLOC], f32r, tag="ezT")
            rinv = apool.tile([1, RLOC], f32, tag="rinv")
            rscr = apool.tile([1, RLOC], f32, tag="rscr")
            rrep = apool.tile([128, RLOC], f32, tag="rrep")
            yT = apool.tile([5, RLOC], f32, tag="yT")
            dpsiT = apool.tile([5, ROWS], f32, tag="dpsiT")
            with tc.tile_pool(name="psum_mlp", bufs=1, space="PSUM") as pmm:
                # column-half outer loop: half 0 runs all layers plus its
                # softmax while the PE streams half 1's layers behind it
                for ci, (c0, cw) in enumerate(CH2):
                    for l in range(DEPTH - 1):
                        src_t, dst_t = hT[l % 2], hT[(l + 1) % 2]
                        for mt in range(KT):
                            ps = pmm.tile([128, 512], f32, tag="mlp",
                                          name="ps", bufs=4)
                            for kt in range(KT):
                                nc.tensor.matmul(
                                    ps[:, 0:cw],
                                    wh_sb[l][:, kt * H + mt * 128:
                                             kt * H + mt * 128 + 128],
                                    src_t[:, kt * RLOC + c0:
                                          kt * RLOC + c0 + cw],
                                    start=(kt == 0), stop=(kt == KT - 1))
                            nc.scalar.activation(
                                dst_t[:, mt * RLOC + c0:mt * RLOC + c0 + cw],
                                ps[:, 0:cw], AF.Tanh,
                                bias=bhp_sb[:, l * KT + mt:l * KT + mt + 1])
                    ps = pmm.tile([5, 512], f32, tag="zed", name="ps", bufs=1)
                    for kt in range(KT):
                        nc.tensor.matmul(
                            ps[:, 0:cw],
                            woutp_sb[:, kt * 5:(kt + 1) * 5],
                            hlast[:, kt * RLOC + c0:kt * RLOC + c0 + cw],
                            start=(kt == 0), stop=(kt == KT - 1))
                    nc.scalar.activation(
                        ezT[:, c0:c0 + cw], ps[:, 0:cw], AF.Exp,
                        bias=bout5_sb[:, 0:1])
                    pss = pmm.tile([1, 512], f32, tag="ssum", name="ps",
                                   bufs=2)
                    nc.tensor.matmul(
                        pss[:, 0:cw], ones5[:], ezT[:, c0:c0 + cw],
                        start=True, stop=True)
                    nc.vector.reciprocal_approx_accurate(
                        rinv[0:1, c0:c0 + cw], pss[0:1, 0:cw],
                        rscr[0:1, c0:c0 + cw])
                    if ci == 0:
                        # replicate off the PE queue so half 1's layer
                        # matmuls are not stalled behind the reciprocal
                        nc.gpsimd.partition_broadcast(
                            rrep[:, c0:c0 + cw], rinv[0:1, c0:c0 + cw])
                        rrep5 = rrep[0:5, c0:c0 + cw]
                    else:
                        # PE is idle after the last half: matmul is fastest
                        psr = pmm.tile([5, 512], f32, tag="rrp", name="ps",
                                       bufs=1)
                        nc.tensor.matmul(
                            psr[:, 0:cw], ones1x5[:], rinv[0:1, c0:c0 + cw],
                            start=True, stop=True)
                        rrep5 = psr[:, 0:cw]
                    nc.vector.tensor_tensor(
                        yT[:, c0:c0 + cw],
                        ezT[:, c0:c0 + cw], rrep5, OP.mult)
                    lo = c0 - 1 if ci else 0
                    hi = c0 + cw - 1
                    nc.vector.tensor_tensor(
                        dpsiT[:, lo:hi], yT[:, lo + 1:hi + 1],
                        yT[:, lo:hi], OP.subtract)

            nc.vector.tensor_scalar(dpsiT[:, ROWS - 1:ROWS],
                                    dpsiT[:, ROWS - 2:ROWS - 1],
                                    lastc5, None, OP.mult)

            # keep PE busy through the DVE softmax tail so the HAM gate
            # stays open for the fold/rev/conv burst
            with tc.tile_pool(name="psum_w3", bufs=1, space="PSUM") as pw3:
                wz3 = pw3.tile([64, 64], f32, tag="warm3")
                for wi in range(50):
                    nc.tensor.matmul(
                        wz3[:], wmm[:, 0:64], wmm[:, 0:64],
                        start=(wi == 0), stop=(wi == 49))

            # ------- fold dpsi only (critical path to the collective) ----
            dloc = spool.tile([128, 40], bf16, tag="dloc")
            yloc = spool.tile([128, 40], f32, tag="yloc")
            with tc.tile_pool(name="psum_fold", bufs=1,
                              space="PSUM") as pf:
                ptd = pf.tile([128, 40], f32, tag="fold")
                for j in range(NQ):
                    nc.tensor.transpose(
                        ptd[:, j * 5:(j + 1) * 5],
                        dpsiT[:, j * 128:(j + 1) * 128],
                        ident5_sb[:],
                    )
                nc.vector.tensor_copy(dloc[:], ptd[:])

                dgr = spool.tile([128, 40], bf16, tag="dgr")
                pr = pf.tile([128, 40], f32, tag="rev")
                nc.tensor.matmul(pr[:], j128_sb[:], dloc[:],
                                 start=True, stop=True)
                nc.vector.tensor_copy(dgr[:], pr[:])
            # ------- local partial Toeplitz conv over all 64 blocks ------
            with tc.tile_pool(name="psum_out", bufs=2, space="PSUM") as po:
                conv = po.tile([128, NB * 5], f32, tag="conv")
                ms = list(range(0, NB, NQ)) + [m for m in range(NB)
                                               if m % NQ != 0]
                for i, m in enumerate(ms):
                    nj = min(NQ, NB - m)
                    nc.tensor.matmul(
                        conv[:, 5 * m:5 * (m + nj)],
                        wmega[:, 128 * m:128 * (m + 1)],
                        dgr[:, 0:5 * nj],
                        start=(i == 0), stop=(i == len(ms) - 1))
                conv_sb = spool.tile([128, NB * 5], ccdt, tag="convsb")
                nc.scalar.copy(conv_sb[:], conv[:])   # idle ACT engine
                nc.gpsimd.dma_start(
                    cc2_in[:].rearrange("(g p) f -> p g f", p=128),
                    conv_sb[:].rearrange("p (g f) -> p g f", g=NCORES))

            # ------- y fold + f (overlap the collective wait) -------
            with tc.tile_pool(name="psum_yfold", bufs=1,
                              space="PSUM") as pfy:
                pty = pfy.tile([128, 40], f32, tag="yfold")
                for j in range(NQ):
                    nc.tensor.transpose(
                        pty[:, j * 5:(j + 1) * 5],
                        yT[:, j * 128:(j + 1) * 128],
                        ident5_sb[:],
                    )
                nc.vector.tensor_copy(yloc[:], pty[:])
            yb4 = yloc[:].rearrange("p (q c) -> p q c", q=NQ)
            fb = spool.tile([128, 40], f32, tag="fb")
            fb4 = fb[:].rearrange("p (q c) -> p q c", q=NQ)
            t1 = spool.tile([128, NQ], f32, tag="t1")
            liv = spool.tile([128, NQ], f32, tag="liv")
            nc.vector.tensor_scalar(liv[:], yb4[:, :, 4], -1.0, 1.0,
                                    OP.mult, OP.add)
            nc.vector.reciprocal(liv[:], liv[:])
            nc.vector.tensor_tensor(t1[:], yb4[:, :, 0], yb4[:, :, 2],
                                    OP.mult)
            nc.vector.tensor_tensor(t1[:], t1[:], liv[:], OP.mult)
            nc.vector.tensor_scalar(t1[:], t1[:], beta128, None, OP.mult)
            nc.vector.tensor_scalar(fb4[:, :, 0], t1[:], -1.0, None,
                                    OP.mult)
            nc.vector.scalar_tensor_tensor(
                fb4[:, :, 1], yb4[:, :, 1], nsig128, t1[:],
                OP.mult, OP.add)
            nc.vector.tensor_scalar(t1[:], yb4[:, :, 1], sig128, None,
                                    OP.mult)
            nc.vector.scalar_tensor_tensor(
                fb4[:, :, 2], yb4[:, :, 2], ngpm128, t1[:],
                OP.mult, OP.add)
            nc.vector.tensor_scalar(fb4[:, :, 3], yb4[:, :, 2], gam128,
                                    None, OP.mult)
            nc.vector.tensor_scalar(fb4[:, :, 4], yb4[:, :, 2], mu128,
                                    None, OP.mult)

            # ------- collective: sum partial convs across cores ----------
            rsb = spool.tile([128, 40], f32, tag="rsb")
            if COLLECTIVE == "RS":
                nc.gpsimd.collective_compute(
                    "ReduceScatter", OP.add,
                    replica_groups=[list(range(NCORES))],
                    ins=[cc2_in[:].opt()], outs=[rs_out[:].opt()])
                nc.gpsimd.dma_start(rsb[:], rs_out[:])
            else:
                nc.gpsimd.collective_compute(
                    "AllToAll", OP.bypass,
                    replica_groups=[list(range(NCORES))],
                    ins=[cc2_in[:].opt()], outs=[a2a_out[:].opt()])
                rsb8 = spool.tile([128, NCORES * 40], ccdt, tag="rsb8")
                nc.gpsimd.dma_start(
                    rsb8[:].rearrange("p (s f) -> p s f", s=NCORES),
                    a2a_out[:].rearrange("(s p) f -> p s f", p=128))
                a1 = spool.tile([128, 160], f32, tag="a1")
                nc.vector.tensor_tensor(a1[:], rsb8[:, 0:160],
                                        rsb8[:, 160:320], OP.add)
                nc.vector.tensor_tensor(a1[:, 0:80], a1[:, 0:80],
                                        a1[:, 80:160], OP.add)
                nc.vector.tensor_tensor(rsb[:], a1[:, 0:40],
                                        a1[:, 40:80], OP.add)

            # ------- residual + per-row partial loss (host reduces) ------
            res = spool.tile([128, 40], f32, tag="res")
            nc.vector.scalar_tensor_tensor(res[:], rsb[:], c128, fb[:],
                                           OP.mult, OP.subtract)
            sq = spool.tile([128, 40], f32, tag="sq")
            rowsum = spool.tile([128, 1], f32, tag="rowsum")
            nc.vector.scalar_tensor_tensor(
                sq[:], res[:], 0.0, res[:], OP.add, OP.mult,
                accum_out=rowsum[:])
            nc.sync.dma_start(out_d.ap(), rowsum[:])

    nc.compile()
    return nc


def _in_maps(inputs):
    t = np.asarray(inputs["t"], np.float32)
    W_in = np.asarray(inputs["W_in"], np.float32)
    b_in = np.asarray(inputs["b_in"], np.float32)
    Wh = np.asarray(inputs["Wh"], np.float32)
    bh = np.asarray(inputs["bh"], np.float32)
    W_out = np.asarray(inputs["W_out"], np.float32)
    b_out = np.asarray(inputs["b_out"], np.float32)

    whp = np.ascontiguousarray(
        Wh.reshape(DEPTH - 1, KT, 128, H).transpose(2, 0, 1, 3)
        .reshape(128, (DEPTH - 1) * KT * H)).astype(np.float16)
    woutp = np.ascontiguousarray(
        W_out.reshape(KT, 128, 5).transpose(1, 0, 2)
        .reshape(128, KT * 5)).astype(np.float16)

    base = np.zeros((128, 44), np.float32)
    base[:, 0:4] = W_in.reshape(KT, 128).T
    base[:, 4:8] = b_in.reshape(KT, 128).T
    base[:, 8:28] = (bh.reshape(DEPTH - 1, KT, 128).transpose(2, 0, 1)
                     .reshape(128, (DEPTH - 1) * KT))
    base[0:5, 28:33] = np.eye(5, dtype=np.float32)
    base[0:5, 33] = b_out
    base[0, 34] = inputs["raw_beta"][0]
    base[0, 35] = inputs["raw_sigma"][0]
    base[0, 36] = inputs["raw_gamma"][0]
    base[0, 37] = inputs["raw_mu"][0]
    base[0, 38] = inputs["z_alpha"][0]

    maps = []
    for d in range(NCORES):
        shift = -128.0 - 1024.0 * d
        smalls = base.copy()
        smalls[0, 40] = shift
        smalls[0, 41] = 1.0 if d < NCORES - 1 else 0.0
        smalls[0, 42] = shift - 1.0
        maps.append({
            "tsh": np.ascontiguousarray(
                np.broadcast_to(t[d * ROWS:(d + 1) * ROWS, 0].reshape(
                    1, RLOC), (128, RLOC))),
            "smalls": smalls,
            "whp": whp,
            "woutp": woutp,
        })
    return maps


def kernel(**inputs) -> np.ndarray:
    from concourse.bass_utils import run_bass_kernel_spmd

    if "nc" not in _CACHE:
        _CACHE["nc"] = _build()
    nc = _CACHE["nc"]
    res = run_bass_kernel_spmd(nc, _in_maps(inputs), list(range(NCORES)))
    total = np.float32(0.0)
    for r in res.results:
        part = np.float32(np.asarray(r["out"], np.float32).sum())
        total = np.float32(total + np.float32(part / np.float32(N * 5)))
    return np.asarray(total, np.float32).reshape(())


# revision 22
# speedup vs baseline: 1.1599x; 1.1122x over previous
"""EpiPINN loss kernel for 8 Trainium2 NeuronCores (Bass/Tile).

Computes: 6-layer tanh MLP (1->512x5->5) over 8192 collocation points,
softmax -> SEIRD components y, Caputo L1 fractional derivative (lower
triangular Toeplitz [8191x8191] @ dpsi), SEIRD residual, scalar MSE loss.

Distribution: data-parallel MLP over rows (1024/core); the Toeplitz matmul
computes per-core partial convolutions for all 64 output blocks from the
local dpsi (core-shifted band wmega); ReduceScatter sums them so core d
receives its own 8 blocks; per-core partial loss summed on host.

v4 schedule: ACT runs one table set (tanh+exp) for the whole kernel.
Layer 0 is an outer product, done as a partition-broadcast DMA of t plus
ACT `scale=`-fused multiplies (no PE, no f32r 2-pass). The Caputo band
(m^e) is evaluated on the DVE with frexp-bitcast + Horner polynomials;
the scalar params (softplus/lgamma/C) run the same polynomials on the
otherwise-idle GPSIMD. PE warm-up bursts use a memset tile (no DMA
dependency) so nothing early ever waits on a slow queue. Collective
payload is f16.
"""

import math

import numpy as np

H = 512
DEPTH = 6
N = 8192
DT = 0.1
MIN_ALPHA = 0.6
NCORES = 8
ROWS = N // NCORES          # 1024 rows per core
NB = N // 128               # 64 global 128-row blocks
NQ = NB // NCORES           # 8 out-blocks per core
WB = 8320                   # wbuf length = 128 * 65  (shifted-kernel values)
WBC = 65                    # wbuf free cols per partition
WMC = 128 * 64              # Wmega columns: diagonals m'' = 0..63
KT = H // 128               # 4 contraction tiles
RLOC = ROWS                 # 1024 rows per core
CH2 = ((0, 512), (512, 512))

COLLECTIVE = "A2A"          # "RS" or "A2A"
CC_DT = "f16"               # collective payload dtype

_CACHE = {}

_SQ2 = math.sqrt(2.0)
_LN2 = math.log(2.0)


def _lgamma_coeffs(deg=7):
    x = np.linspace(1.0, 1.4, 2001)
    y = np.array([math.lgamma(v) for v in x])
    return np.polyfit(x, y, deg)


def _ln_coeffs(deg=10):
    # ln(1+t) on [sqrt(2)/2-1, sqrt(2)-1]; max abs err ~2.4e-9
    t = np.linspace(_SQ2 / 2 - 1, _SQ2 - 1, 20001)
    return np.polyfit(t, np.log1p(t), deg)


def _exp_coeffs(deg=12):
    # e^x on [0, 3.65] centered at 1.825; max rel err ~2e-9
    x = np.linspace(0.0, 3.65, 20001)
    return np.polyfit(x - 1.825, np.exp(x), deg)


def _build():
    import concourse.bass as bass
    import concourse.tile as tile
    from concourse import bacc, mybir
    from ml_dtypes import bfloat16 as ml_bf16

    f32 = mybir.dt.float32
    bf16 = mybir.dt.bfloat16
    f16 = mybir.dt.float16
    i32 = mybir.dt.int32
    f32r = mybir.dt.float32r
    AF = mybir.ActivationFunctionType
    OP = mybir.AluOpType
    ccdt = f16 if CC_DT == "f16" else f32

    nc = bacc.Bacc("TRN2", target_bir_lowering=False, debug=False,
                   num_devices=NCORES)

    # ---- kernel I/O ----
    tsh = nc.dram_tensor("tsh", [128, RLOC], f32, kind="ExternalInput")
    # all small inputs packed into one tensor -> one DMA, one descgen
    smalls = nc.dram_tensor("smalls", [128, 44], f32, kind="ExternalInput")
    whp = nc.dram_tensor("whp", [128, (DEPTH - 1) * KT * H], f16,
                         kind="ExternalInput")
    woutp = nc.dram_tensor("woutp", [128, KT * 5], f16, kind="ExternalInput")
    out_d = nc.dram_tensor("out", [128, 1], f32, kind="ExternalOutput")

    j128_d = nc.inline_tensor(
        np.eye(128, dtype=np.float32)[::-1].copy().astype(ml_bf16),
        name="j128")

    lg = _lgamma_coeffs()
    lnc = _ln_coeffs()
    exc = _exp_coeffs()

    def ln_dve(eng, pool, out, x_ap, pcount, cols, tagp):
        """out = ln(x) for positive f32 x via frexp bitcast + Horner."""
        xi = x_ap.bitcast(i32)
        fi = pool.tile([pcount, cols], i32, tag=tagp + "fi")
        eng.tensor_scalar(fi[:], xi, 0x7FFFFF, 0x3F800000,
                          OP.bitwise_and, OP.bitwise_or)
        f = fi[:].bitcast(f32)
        ei = pool.tile([pcount, cols], i32, tag=tagp + "ei")
        eng.tensor_scalar(ei[:], xi, 0x7F800000, None, OP.bitwise_and)
        ef = pool.tile([pcount, cols], f32, tag=tagp + "ef")
        eng.tensor_copy(ef[:], ei[:])           # exact int -> f32
        eng.tensor_scalar(ef[:], ef[:], 2.0 ** -23, -127.0,
                          OP.mult, OP.add)
        msk = pool.tile([pcount, cols], f32, tag=tagp + "mk")
        eng.tensor_scalar(msk[:], f, -_SQ2, 1e30, OP.add, OP.mult)
        eng.tensor_scalar(msk[:], msk[:], 0.0, 1.0, OP.max, OP.min)
        # e2 = ef + msk ; f2 = f * (1 - 0.5*msk) ; t = f2 - 1
        eng.tensor_tensor(ef[:], ef[:], msk[:], OP.add)
        eng.tensor_scalar(msk[:], msk[:], -0.5, 1.0, OP.mult, OP.add)
        t = pool.tile([pcount, cols], f32, tag=tagp + "t")
        eng.tensor_tensor(t[:], f, msk[:], OP.mult)
        eng.tensor_scalar_add(t[:], t[:], -1.0)
        p = pool.tile([pcount, cols], f32, tag=tagp + "p")
        eng.memset(p[:], float(lnc[0]))
        for c in lnc[1:]:
            eng.tensor_tensor(p[:], p[:], t[:], OP.mult)
            eng.tensor_scalar_add(p[:], p[:], float(c))
        eng.scalar_tensor_tensor(out, ef[:], _LN2, p[:], OP.mult, OP.add)

    def exp_dve(eng, pool, out, x_ap, pcount, cols, tagp):
        """out = exp(x) for x in [0, 3.65] via centered Horner."""
        u = pool.tile([pcount, cols], f32, tag=tagp + "u")
        eng.tensor_scalar_add(u[:], x_ap, -1.825)
        p = pool.tile([pcount, cols], f32, tag=tagp + "q")
        eng.memset(p[:], float(exc[0]))
        for c in exc[1:-1]:
            eng.tensor_tensor(p[:], p[:], u[:], OP.mult)
            eng.tensor_scalar_add(p[:], p[:], float(c))
        eng.tensor_tensor(p[:], p[:], u[:], OP.mult)
        eng.tensor_scalar(out, p[:], float(exc[-1]), None, OP.add)

    with tile.TileContext(nc, num_cores=NCORES) as tc:
        with (
            tc.tile_pool(name="dram", bufs=1, space="DRAM") as dram,
            tc.tile_pool(name="const", bufs=1) as cpool,
            tc.tile_pool(name="acts", bufs=1) as apool,
            tc.tile_pool(name="small", bufs=1) as spool,
        ):
            # ------- DRAM scratch -------
            wbuf_dram = dram.tile([WB], bf16)
            cc2_in = dram.tile([128 * NCORES, 40], ccdt)
            rs_out = dram.tile([128, 40], ccdt)
            if COLLECTIVE == "A2A":
                a2a_out = dram.tile([128 * NCORES, 40], ccdt)

            # ------- DMAs: smalls packed on sync; weights on the scalar
            # queue; t-broadcast on gpsimd -------
            sm = cpool.tile([128, 44], f32)
            nc.sync.dma_start(sm[:], smalls.ap())
            winp_sb = sm[:, 0:4]
            binp_sb = sm[:, 4:8]
            bhp_sb = sm[:, 8:28]
            ident5_sb = sm[0:5, 28:33]
            bout5_sb = sm[0:5, 33:34]
            par_sb = sm[0:1, 34:42]
            coref_sb = sm[0:1, 40:44]
            j128_sb = cpool.tile([128, 128], bf16)
            nc.sync.dma_start(j128_sb[:], j128_d.ap())

            # t pre-broadcast host-side (layer-0 moving operand)
            tb = cpool.tile([128, RLOC], f32)
            nc.sync.dma_start(tb[:], tsh.ap())

            whp_sb = cpool.tile([128, (DEPTH - 1) * KT * H], f16)
            nc.scalar.dma_start(whp_sb[:], whp.ap())
            wh_sb = [whp_sb[:, l * KT * H:(l + 1) * KT * H]
                     for l in range(DEPTH - 1)]
            woutp_sb = cpool.tile([128, KT * 5], f16)
            nc.scalar.dma_start(woutp_sb[:], woutp.ap())

            # memset consts (DVE, ready instantly)
            wmm = cpool.tile([128, 64], bf16)
            nc.vector.memset(wmm[:], 0.25)
            ones5f = cpool.tile([5, 1], f32)
            nc.vector.memset(ones5f[:], 1.0)
            ones5 = cpool.tile([5, 1], f32r)
            nc.vector.tensor_copy(ones5[:], ones5f[:])
            ones1x5 = cpool.tile([1, 5], f32)
            nc.vector.memset(ones1x5[:], 1.0)
            ones128t = cpool.tile([1, 128], f32)
            nc.vector.memset(ones128t[:], 1.0)
            ones128 = cpool.tile([128, 1], f32)
            nc.vector.memset(ones128[:], 1.0)

            # ------- params: the only ACT use outside tanh/exp stream ----
            sp_e = spool.tile([1, 8], f32, tag="sp")
            nc.scalar.activation(sp_e[0:1, 0:4], par_sb[0:1, 0:4], AF.Exp)
            alp = spool.tile([1, 4], f32, tag="alp")
            nc.scalar.activation(alp[0:1, 0:1], par_sb[0:1, 4:5], AF.Exp,
                                 scale=-1.0)
            # alpha chain on DVE (fast, feeds ebp)
            nc.vector.tensor_scalar_add(alp[0:1, 0:1], alp[0:1, 0:1], 1.0)
            nc.vector.reciprocal(alp[0:1, 1:2], alp[0:1, 0:1])
            nc.vector.tensor_scalar(alp[0:1, 2:3], alp[0:1, 1:2],
                                    1.0 - MIN_ALPHA, MIN_ALPHA,
                                    OP.mult, OP.add)
            nc.vector.tensor_scalar(alp[0:1, 3:4], alp[0:1, 2:3],
                                    -1.0, 1.0, OP.mult, OP.add)  # e = 1-a

            e2 = spool.tile([1, 4], f32, tag="e2")
            nc.vector.tensor_copy(e2[0:1, 0:1], alp[0:1, 3:4])
            nc.vector.tensor_copy(e2[0:1, 1:3], coref_sb[0:1, 0:2])
            nc.vector.tensor_copy(e2[0:1, 3:4], coref_sb[0:1, 2:3])
            eb = cpool.tile([128, 4], f32)
            with tc.tile_pool(name="psum_pre", bufs=1, space="PSUM") as peb:
                ebp = peb.tile([128, 4], f32, tag="ebp")
                nc.tensor.matmul(ebp[:], ones128t[:], e2[0:1, :],
                                 start=True, stop=True)
                nc.vector.tensor_copy(eb[:], ebp[:])
            e128 = eb[:, 0:1]
            shiftm1 = eb[:, 3:4]
            lastc5 = eb[0:5, 2:3]

            # ------- Caputo band on DVE -------
            wtmp = tc.tile_pool(name="wtmp", bufs=1)
            with wtmp as wt:
                vi = wt.tile([128, 66], i32, tag="vi")
                nc.gpsimd.iota(vi[:], [[1, 66]], channel_multiplier=65)
                qraw = wt.tile([128, 66], f32, tag="qraw")
                nc.vector.tensor_copy(qraw[:], vi[:])
                nc.vector.tensor_scalar(qraw[:], qraw[:], shiftm1, None,
                                        OP.add)
                mk1 = wt.tile([128, WBC], f32, tag="mk1")
                nc.vector.tensor_scalar(mk1[:], qraw[:, 0:WBC], 1.0, None,
                                        OP.add)
                nc.vector.tensor_scalar(mk1[:], mk1[:], 0.0, 1.0, OP.max,
                                        OP.min)
                mk2 = wt.tile([128, WBC], f32, tag="mk2")
                nc.vector.tensor_scalar(mk2[:], qraw[:, 0:WBC], 0.0, 1.0,
                                        OP.max, OP.min)
                mk3 = wt.tile([128, WBC], f32, tag="mk3")
                nc.vector.tensor_scalar(mk3[:], qraw[:, 0:WBC], -1.0, 8191.0,
                                        OP.mult, OP.add)
                nc.vector.tensor_scalar(mk3[:], mk3[:], 0.0, 1.0, OP.max,
                                        OP.min)
                qc = wt.tile([128, 66], f32, tag="qc")
                nc.vector.tensor_scalar(qc[:], qraw[:], 1.0, None, OP.max)
                g = wt.tile([128, 66], f32, tag="g")
                ln_dve(nc.vector, wt, g[:], qc[:], 128, 66, "L")
                nc.vector.tensor_scalar(g[:], g[:], e128, None, OP.mult)
                ee = wt.tile([128, 66], f32, tag="ee")
                exp_dve(nc.vector, wt, ee[:], g[:], 128, 66, "X")
                w1 = wt.tile([128, WBC], f32, tag="w1")
                nc.vector.tensor_tensor(w1[:], ee[:, 1:66], mk1[:], OP.mult)
                nc.vector.tensor_tensor(mk2[:], ee[:, 0:WBC], mk2[:],
                                        OP.mult)
                nc.vector.tensor_tensor(w1[:], w1[:], mk2[:], OP.subtract)
                nc.vector.tensor_tensor(w1[:], w1[:], mk3[:], OP.mult)
                wbf = wt.tile([128, WBC], bf16, tag="wbf")
                nc.vector.tensor_copy(wbf[:], w1[:])
                nc.sync.dma_start(
                    wbuf_dram[:].rearrange("(p f) -> p f", p=128), wbf[:])

            wmega = cpool.tile([128, WMC], bf16)
            src = bass.AP(
                tensor=wbuf_dram[:].tensor, offset=1,
                ap=[[1, 128], [1, WMC]])
            nc.sync.dma_start(wmega[:], src)

            # ------- softplus / lgamma / C on DVE, after the band work ---
            ve = nc.vector
            ve.tensor_scalar_add(sp_e[0:1, 0:4], sp_e[0:1, 0:4], 1.0)
            sp = spool.tile([1, 8], f32, tag="sp2")
            ln_dve(ve, spool, sp[0:1, 0:4], sp_e[0:1, 0:4], 1, 4, "S")
            lgm = spool.tile([1, 2], f32, tag="lgm")
            ve.tensor_scalar_add(lgm[0:1, 1:2], alp[0:1, 3:4], 1.0)
            ve.memset(lgm[0:1, 0:1], float(lg[0]))
            for k in range(1, len(lg)):
                ve.tensor_tensor(lgm[0:1, 0:1], lgm[0:1, 0:1],
                                 lgm[0:1, 1:2], OP.mult)
                ve.tensor_scalar_add(lgm[0:1, 0:1], lgm[0:1, 0:1],
                                     float(lg[k]))
            cc_s = spool.tile([1, 2], f32, tag="ccs")
            ve.scalar_tensor_tensor(
                cc_s[0:1, 0:1], alp[0:1, 2:3], -math.log(DT), lgm[0:1, 0:1],
                OP.mult, OP.subtract)
            exp_dve(ve, spool, cc_s[0:1, 1:2], cc_s[0:1, 0:1], 1, 1, "C")

            sc16 = spool.tile([1, 16], f32, tag="sc16")
            ve.tensor_copy(sc16[0:1, 0:4], sp[0:1, 0:4])
            ve.tensor_tensor(sc16[0:1, 4:5], sp[0:1, 2:3],
                             sp[0:1, 3:4], OP.add)
            ve.tensor_scalar(sc16[0:1, 5:6], sp[0:1, 1:2], -1.0, None,
                             OP.mult)
            ve.tensor_scalar(sc16[0:1, 6:7], sc16[0:1, 4:5], -1.0, None,
                             OP.mult)
            ve.tensor_copy(sc16[0:1, 7:8], cc_s[0:1, 1:2])
            scb = cpool.tile([128, 8], f32)
            nc.gpsimd.partition_broadcast(scb[:], sc16[0:1, 0:8])
            beta128 = scb[:, 0:1]
            sig128 = scb[:, 1:2]
            gam128 = scb[:, 2:3]
            mu128 = scb[:, 3:4]
            nsig128 = scb[:, 5:6]
            ngpm128 = scb[:, 6:7]
            c128 = scb[:, 7:8]

            # ------- MLP: L0 fused into ACT via scale=; L1+ on PE -------
            hT = [apool.tile([128, KT * RLOC], f16, tag="hA", name="hA"),
                  apool.tile([128, KT * RLOC], f16, tag="hB", name="hB")]
            for mt in range(KT):
                for c0, cw in CH2:
                    nc.scalar.activation(
                        hT[0][:, mt * RLOC + c0:mt * RLOC + c0 + cw],
                        tb[:, c0:c0 + cw], AF.Tanh,
                        scale=winp_sb[:, mt:mt + 1],
                        bias=binp_sb[:, mt:mt + 1])
            # PE warm-up right before the hidden-layer stream so the HAM
            # clock gate is released when L1 starts (memset tile: no DMA dep)
            with tc.tile_pool(name="psum_warm", bufs=1, space="PSUM") as pw:
                warm = pw.tile([64, 64], f32, tag="warm")
                for wi in range(45):
                    nc.tensor.matmul(
                        warm[:], wmm[:, 0:64], wmm[:, 0:64],
                        start=(wi == 0), stop=(wi == 44))
            hlast = hT[(DEPTH - 1) % 2]
            ezT = apool.tile([5, RLOC], f32r, tag="ezT")
            rinv = apool.tile([1, RLOC], f32, tag="rinv")
            rscr = apool.tile([1, RLOC], f32, tag="rscr")
            rrep = apool.tile([128, RLOC], f32, tag="rrep")
            yT = apool.tile([5, RLOC], f32, tag="yT")
            dpsiT = apool.tile([5, ROWS], f32, tag="dpsiT")
            with tc.tile_pool(name="psum_mlp", bufs=1, space="PSUM") as pmm:
                # column-half outer loop: half 0 runs all layers plus its
                # softmax while the PE streams half 1's layers behind it
                for ci, (c0, cw) in enumerate(CH2):
                    for l in range(DEPTH - 1):
                        src_t, dst_t = hT[l % 2], hT[(l + 1) % 2]
                        for mt in range(KT):
                            ps = pmm.tile([128, 512], f32, tag="mlp",
                                          name="ps", bufs=4)
                            for kt in range(KT):
                                nc.tensor.matmul(
                                    ps[:, 0:cw],
                                    wh_sb[l][:, kt * H + mt * 128:
                                             kt * H + mt * 128 + 128],
                                    src_t[:, kt * RLOC + c0:
                                          kt * RLOC + c0 + cw],
                                    start=(kt == 0), stop=(kt == KT - 1))
                            nc.scalar.activation(
                                dst_t[:, mt * RLOC + c0:mt * RLOC + c0 + cw],
                                ps[:, 0:cw], AF.Tanh,
                                bias=bhp_sb[:, l * KT + mt:l * KT + mt + 1])
                    ps = pmm.tile([5, 512], f32, tag="zed", name="ps", bufs=1)
                    for kt in range(KT):
                        nc.tensor.matmul(
                            ps[:, 0:cw],
                            woutp_sb[:, kt * 5:(kt + 1) * 5],
                            hlast[:, kt * RLOC + c0:kt * RLOC + c0 + cw],
                            start=(kt == 0), stop=(kt == KT - 1))
                    nc.scalar.activation(
                        ezT[:, c0:c0 + cw], ps[:, 0:cw], AF.Exp,
                        bias=bout5_sb[:, 0:1])
                    pss = pmm.tile([1, 512], f32, tag="ssum", name="ps",
                                   bufs=2)
                    nc.tensor.matmul(
                        pss[:, 0:cw], ones5[:], ezT[:, c0:c0 + cw],
                        start=True, stop=True)
                    nc.vector.reciprocal_approx_accurate(
                        rinv[0:1, c0:c0 + cw], pss[0:1, 0:cw],
                        rscr[0:1, c0:c0 + cw])
                    if ci == 0:
                        # replicate off the PE queue so half 1's layer
                        # matmuls are not stalled behind the reciprocal
                        nc.gpsimd.partition_broadcast(
                            rrep[:, c0:c0 + cw], rinv[0:1, c0:c0 + cw])
                        rrep5 = rrep[0:5, c0:c0 + cw]
                    else:
                        # PE is idle after the last half: matmul is fastest
                        psr = pmm.tile([5, 512], f32, tag="rrp", name="ps",
                                       bufs=1)
                        nc.tensor.matmul(
                            psr[:, 0:cw], ones1x5[:], rinv[0:1, c0:c0 + cw],
                            start=True, stop=True)
                        rrep5 = psr[:, 0:cw]
                    nc.vector.tensor_tensor(
                        yT[:, c0:c0 + cw],
                        ezT[:, c0:c0 + cw], rrep5, OP.mult)
                    lo = c0 - 1 if ci else 0
                    hi = c0 + cw - 1
                    nc.vector.tensor_tensor(
                        dpsiT[:, lo:hi], yT[:, lo + 1:hi + 1],
                        yT[:, lo:hi], OP.subtract)

            nc.vector.tensor_scalar(dpsiT[:, ROWS - 1:ROWS],
                                    dpsiT[:, ROWS - 2:ROWS - 1],
                                    lastc5, None, OP.mult)

            # keep PE busy through the DVE softmax tail so the HAM gate
            # stays open for the fold/rev/conv burst
            with tc.tile_pool(name="psum_w3", bufs=1, space="PSUM") as pw3:
                wz3 = pw3.tile([64, 64], f32, tag="warm3")
                for wi in range(25):
                    nc.tensor.matmul(
                        wz3[:], wmm[:, 0:64], wmm[:, 0:64],
                        start=(wi == 0), stop=(wi == 24))

            # ------- fold dpsi only (critical path to the collective) ----
            dloc = spool.tile([128, 40], bf16, tag="dloc")
            yloc = spool.tile([128, 40], f32, tag="yloc")
            with tc.tile_pool(name="psum_fold", bufs=1,
                              space="PSUM") as pf:
                ptd = pf.tile([128, 40], f32, tag="fold")
                for j in range(NQ):
                    nc.tensor.transpose(
                        ptd[:, j * 5:(j + 1) * 5],
                        dpsiT[:, j * 128:(j + 1) * 128],
                        ident5_sb[:],
                    )
                nc.vector.tensor_copy(dloc[:], ptd[:])

                dgr = spool.tile([128, 40], bf16, tag="dgr")
                pr = pf.tile([128, 40], f32, tag="rev")
                nc.tensor.matmul(pr[:], j128_sb[:], dloc[:],
                                 start=True, stop=True)
                nc.vector.tensor_copy(dgr[:], pr[:])
            # ------- local partial Toeplitz conv over all 64 blocks ------
            with tc.tile_pool(name="psum_out", bufs=2, space="PSUM") as po:
                conv = po.tile([128, NB * 5], f32, tag="conv")
                ms = list(range(0, NB, NQ)) + [m for m in range(NB)
                                               if m % NQ != 0]
                for i, m in enumerate(ms):
                    nj = min(NQ, NB - m)
                    nc.tensor.matmul(
                        conv[:, 5 * m:5 * (m + nj)],
                        wmega[:, 128 * m:128 * (m + 1)],
                        dgr[:, 0:5 * nj],
                        start=(i == 0), stop=(i == len(ms) - 1))
                conv_sb = spool.tile([128, NB * 5], ccdt, tag="convsb")
                nc.scalar.copy(conv_sb[:], conv[:])   # idle ACT engine
                nc.gpsimd.dma_start(
                    cc2_in[:].rearrange("(g p) f -> p g f", p=128),
                    conv_sb[:].rearrange("p (g f) -> p g f", g=NCORES))

            # ------- y fold + f (overlap the collective wait) -------
            with tc.tile_pool(name="psum_yfold", bufs=1,
                              space="PSUM") as pfy:
                pty = pfy.tile([128, 40], f32, tag="yfold")
                for j in range(NQ):
                    nc.tensor.transpose(
                        pty[:, j * 5:(j + 1) * 5],
                        yT[:, j * 128:(j + 1) * 128],
                        ident5_sb[:],
                    )
                nc.vector.tensor_copy(yloc[:], pty[:])
            yb4 = yloc[:].rearrange("p (q c) -> p q c", q=NQ)
            fb = spool.tile([128, 40], f32, tag="fb")
            fb4 = fb[:].rearrange("p (q c) -> p q c", q=NQ)
            t1 = spool.tile([128, NQ], f32, tag="t1")
            liv = spool.tile([128, NQ], f32, tag="liv")
            nc.vector.tensor_scalar(liv[:], yb4[:, :, 4], -1.0, 1.0,
                                    OP.mult, OP.add)
            nc.vector.reciprocal(liv[:], liv[:])
            nc.vector.tensor_tensor(t1[:], yb4[:, :, 0], yb4[:, :, 2],
                                    OP.mult)
            nc.vector.tensor_tensor(t1[:], t1[:], liv[:], OP.mult)
            nc.vector.tensor_scalar(t1[:], t1[:], beta128, None, OP.mult)
            nc.vector.tensor_scalar(fb4[:, :, 0], t1[:], -1.0, None,
                                    OP.mult)
            nc.vector.scalar_tensor_tensor(
                fb4[:, :, 1], yb4[:, :, 1], nsig128, t1[:],
                OP.mult, OP.add)
            nc.vector.tensor_scalar(t1[:], yb4[:, :, 1], sig128, None,
                                    OP.mult)
            nc.vector.scalar_tensor_tensor(
                fb4[:, :, 2], yb4[:, :, 2], ngpm128, t1[:],
                OP.mult, OP.add)
            nc.vector.tensor_scalar(fb4[:, :, 3], yb4[:, :, 2], gam128,
                                    None, OP.mult)
            nc.vector.tensor_scalar(fb4[:, :, 4], yb4[:, :, 2], mu128,
                                    None, OP.mult)

            # ------- collective: sum partial convs across cores ----------
            rsb = spool.tile([128, 40], f32, tag="rsb")
            if COLLECTIVE == "RS":
                nc.gpsimd.collective_compute(
                    "ReduceScatter", OP.add,
                    replica_groups=[list(range(NCORES))],
                    ins=[cc2_in[:].opt()], outs=[rs_out[:].opt()])
                nc.gpsimd.dma_start(rsb[:], rs_out[:])
            else:
                nc.gpsimd.collective_compute(
                    "AllToAll", OP.bypass,
                    replica_groups=[list(range(NCORES))],
                    ins=[cc2_in[:].opt()], outs=[a2a_out[:].opt()])
                rsb8 = spool.tile([128, NCORES * 40], ccdt, tag="rsb8")
                nc.gpsimd.dma_start(
                    rsb8[:].rearrange("p (s f) -> p s f", s=NCORES),
                    a2a_out[:].rearrange("(s p) f -> p s f", p=128))
                a1 = spool.tile([128, 160], f32, tag="a1")
                nc.vector.tensor_tensor(a1[:], rsb8[:, 0:160],
                                        rsb8[:, 160:320], OP.add)
                nc.vector.tensor_tensor(a1[:, 0:80], a1[:, 0:80],
                                        a1[:, 80:160], OP.add)
                nc.vector.tensor_tensor(rsb[:], a1[:, 0:40],
                                        a1[:, 40:80], OP.add)

            # ------- residual + per-row partial loss (host reduces) ------
            res = spool.tile([128, 40], f32, tag="res")
            nc.vector.scalar_tensor_tensor(res[:], rsb[:], c128, fb[:],
                                           OP.mult, OP.subtract)
            sq = spool.tile([128, 40], f32, tag="sq")
            rowsum = spool.tile([128, 1], f32, tag="rowsum")
            nc.vector.scalar_tensor_tensor(
                sq[:], res[:], 0.0, res[:], OP.add, OP.mult,
                accum_out=rowsum[:])
            nc.sync.dma_start(out_d.ap(), rowsum[:])

    nc.compile()
    return nc


def _in_maps(inputs):
    t = np.asarray(inputs["t"], np.float32)
    W_in = np.asarray(inputs["W_in"], np.float32)
    b_in = np.asarray(inputs["b_in"], np.float32)
    Wh = np.asarray(inputs["Wh"], np.float32)
    bh = np.asarray(inputs["bh"], np.float32)
    W_out = np.asarray(inputs["W_out"], np.float32)
    b_out = np.asarray(inputs["b_out"], np.float32)

    whp = np.ascontiguousarray(
        Wh.reshape(DEPTH - 1, KT, 128, H).transpose(2, 0, 1, 3)
        .reshape(128, (DEPTH - 1) * KT * H)).astype(np.float16)
    woutp = np.ascontiguousarray(
        W_out.reshape(KT, 128, 5).transpose(1, 0, 2)
        .reshape(128, KT * 5)).astype(np.float16)

    base = np.zeros((128, 44), np.float32)
    base[:, 0:4] = W_in.reshape(KT, 128).T
    base[:, 4:8] = b_in.reshape(KT, 128).T
    base[:, 8:28] = (bh.reshape(DEPTH - 1, KT, 128).transpose(2, 0, 1)
                     .reshape(128, (DEPTH - 1) * KT))
    base[0:5, 28:33] = np.eye(5, dtype=np.float32)
    base[0:5, 33] = b_out
    base[0, 34] = inputs["raw_beta"][0]
    base[0, 35] = inputs["raw_sigma"][0]
    base[0, 36] = inputs["raw_gamma"][0]
    base[0, 37] = inputs["raw_mu"][0]
    base[0, 38] = inputs["z_alpha"][0]

    maps = []
    for d in range(NCORES):
        shift = -128.0 - 1024.0 * d
        smalls = base.copy()
        smalls[0, 40] = shift
        smalls[0, 41] = 1.0 if d < NCORES - 1 else 0.0
        smalls[0, 42] = shift - 1.0
        maps.append({
            "tsh": np.ascontiguousarray(
                np.broadcast_to(t[d * ROWS:(d + 1) * ROWS, 0].reshape(
                    1, RLOC), (128, RLOC))),
            "smalls": smalls,
            "whp": whp,
            "woutp": woutp,
        })
    return maps


def kernel(**inputs) -> np.ndarray:
    from concourse.bass_utils import run_bass_kernel_spmd

    if "nc" not in _CACHE:
        _CACHE["nc"] = _build()
    nc = _CACHE["nc"]
    res = run_bass_kernel_spmd(nc, _in_maps(inputs), list(range(NCORES)))
    total = np.float32(0.0)
    for r in res.results:
        part = np.float32(np.asarray(r["out"], np.float32).sum())
        total = np.float32(total + np.float32(part / np.float32(N * 5)))
    return np.asarray(total, np.float32).reshape(())
